# revision 8
# baseline (speedup 1.0000x reference)
# RBF Gram matrix kernel for Trainium2 (8 NeuronCores, SPMD).
#
# reference:  G[i, j] = exp(-gamma * ||x_i - y_j||^2)
#
# Factorized form (per element):
#   G[i, j] = exp(2*gamma*xy[i,j] - gamma*||x_i||^2) * exp(-gamma*||y_j||^2)
#           =                E[i, j]                 *        cy[j]
# The exp(...) argument 2g*xy - g*x2 is <= ~-60 for this data regime
# (and in general <= g*(y2_j - ||x_i-y_j||^2)), so E never overflows; the
# per-column cy factor is applied after the exp as a bf16 multiply, which
# runs on DVE in 2x mode (SBUF/bf16) instead of the 1x fp32 PSUM add.
#
# Sharding: row-shard x across 8 cores (1024 rows each), replicate y.
# Each core computes a [1024, 8192] slice of G:
#   PE   : xy = x_c @ y^T    fp8(e4m3) DoubleRow matmuls: K=256/instr,
#          N=512, weights reused across 4 moving streams
#   ACT  : E = Exp(2g*xy + (-g*||x||^2))  [128,2048] straight from PSUM
#   DVE  : o = E * cy        bf16 2x mode, cy pre-broadcast from host
#   DMA  : o tile (bf16) -> DRAM; host upcasts to fp32
#
# Loop order: q (column pair, 2048 wide) outer, m (row tile) inner, so the
# first y blocks unlock full-rate PE after ~1MB of input DMA.
import os

import numpy as np
import ml_dtypes

N_CORES = 8
N_FULL = 8192          # rows of x (and of G)
M_FULL = 8192          # rows of y (cols of G)
D = 512                # feature dim (contraction)
MC = N_FULL // N_CORES # 1024 rows of x per core
P = 128                # SBUF partitions
NT = 512               # matmul moving-dim tile (one fp32 psum bank)
KT = D // P            # 4 k-tiles of 128 (2 DoubleRow pairs)
MT = MC // P           # 8 m-tiles per core
PRW = 2048             # ACT/DVE/DMA-out chunk width (one "pair" = 2 ngroups)
NB = M_FULL // NT      # 16 y blocks
NQ = M_FULL // PRW     # 4 column pairs

_cache = {}


def _build_program_raw(scale2g: float, mc: int, n_full: int, d: int):
    """Raw-Bass build: explicit per-engine programs + hand-rolled semaphores."""
    from contextlib import ExitStack, contextmanager

    import concourse.bass as bass
    import concourse.mybir as mybir
    from concourse import bacc

    class _NoBarrierBlock(bass.BassBlock):
        """BassBlock whose exit emits per-engine drains but no all-engine
        barrier; cross-engine ordering is fully covered by our semaphores."""

        def __exit__(self, exc_type, exc_val, exc_tb):
            if exc_type is not None:
                return
            for engine, last_body in self.last_body.items():
                with self.bass.body(last_body, parent=self.bass.cur_bb,
                                    allow_existing_parent=True):
                    engine.br(self.end_bb)
            self.bass.switch_bb(self.end_bb)
            gpsimd_type = self.bass.gpsimd.engine
            for eng_type, eng in self.bass.engines.items():
                if eng_type == gpsimd_type:
                    continue
                dr = mybir.InstDrain(
                    name=self.bass.get_next_instruction_name(),
                    ins=[], outs=[], bass_is_fusable=False)
                dr.engine = eng_type
                eng.add_instruction(dr)

    @contextmanager
    def _no_barrier_block(nc):
        assert nc.cur_block is None
        blk = _NoBarrierBlock(nc, f"block_{nc.next_id()}")
        nc.cur_block = blk
        try:
            with blk:
                yield blk
        finally:
            nc.cur_block = None

    DR = mybir.MatmulPerfMode.DoubleRow
    f8 = mybir.dt.float8e4
    bf16 = mybir.dt.bfloat16
    fp32 = mybir.dt.float32

    mt = mc // P                 # 8
    kt = d // P                  # 4
    nb = n_full // NT            # 16
    nq = n_full // PRW           # 4
    GQ = nq * mt                 # 32 pairs
    ES = 4                       # E staging slots of [128, PRW]
    OS = 6                       # out staging slots of [128, PRW]
    NWARM = int(os.environ.get("RBF_NWARM", "24"))
    SPLIT = 4                    # tail split of last pair (512-wide chunks)

    nc = bacc.Bacc("TRN2", target_bir_lowering=False, debug=False,
                   num_devices=N_CORES)

    xT_d = nc.dram_tensor("xTb", [P, mt * kt, P], f8,
                          kind="ExternalInput").ap()
    yT_d = nc.dram_tensor("yTb", [P, nb * kt, NT], f8,
                          kind="ExternalInput").ap()
    cy_d = nc.dram_tensor("cyb", [P, n_full], bf16,
                          kind="ExternalInput").ap()
    x2_d = nc.dram_tensor("x2b", [P, mt], fp32,
                          kind="ExternalInput").ap()
    out_d = nc.dram_tensor("out", [mc, n_full], bf16,
                           kind="ExternalOutput").ap()

    with ExitStack() as ctx:
        ec = ctx.enter_context
        xT_sb = ec(nc.sbuf_tensor([P, mt * kt, P], f8))
        yT_sb = ec(nc.sbuf_tensor([P, nb * kt, NT], f8))
        cy_sb = ec(nc.sbuf_tensor([P, n_full], bf16))
        x2_sb = ec(nc.sbuf_tensor([P, mt], fp32))
        scr_sb = ec(nc.sbuf_tensor([P, 2, P], f8))
        e_sb = ec(nc.sbuf_tensor([P, ES * PRW], bf16))
        o_sb = ec(nc.sbuf_tensor([P, OS * PRW], bf16))
        ps = ec(nc.psum_tensor([P, 2 * PRW], fp32))

        s_scr = ec(nc.semaphore(name="s_scr"))
        s_xT = [ec(nc.semaphore(name=f"s_xT{i}")) for i in range(3)]
        s_yb = [ec(nc.semaphore(name=f"s_yb{i}")) for i in range(nb)]
        yb_need = [0] * nb
        s_cy = [ec(nc.semaphore(name=f"s_cy{i}")) for i in range(nq)]
        s_x2 = ec(nc.semaphore(name="s_x2"))
        s_mm = ec(nc.semaphore(name="s_mm"))
        s_act = ec(nc.semaphore(name="s_act"))
        s_dve = ec(nc.semaphore(name="s_dve"))
        s_osl = [ec(nc.semaphore(name=f"s_osl{i}")) for i in range(OS)]

        def lhsT(m, kp):
            return xT_sb[:, m * kt + 2 * kp:m * kt + 2 * kp + 2, :]

        def rhs(b, kp):
            return yT_sb[:, b * kt + 2 * kp:b * kt + 2 * kp + 2, :]

        with _no_barrier_block(nc) as block:

            osl_cnt = [0] * OS       # total incs emitted per out-slot sem
            osl_wait = {}            # gq -> count vector must wait for

            @block.sync
            def _(sync):
                # startup set, in critical-path order. First y block split
                # 8-way (per k-tile, per half-NT) to ride parallel queues.
                for k in range(kt):
                    for h in range(2):
                        sync.dma_start(
                            out=yT_sb[:, k:k + 1, h * (NT // 2):(h + 1) * (NT // 2)],
                            in_=yT_d[:, k:k + 1, h * (NT // 2):(h + 1) * (NT // 2)]
                        ).then_inc(s_yb[0], 16)
                yb_need[0] = 16 * kt * 2
                for h in range(2):
                    sync.dma_start(out=xT_sb[:, h * 2:(h + 1) * 2, :],
                                   in_=xT_d[:, h * 2:(h + 1) * 2, :]
                                   ).then_inc(s_xT[0], 16)
                sync.dma_start(out=x2_sb[:], in_=x2_d).then_inc(s_x2, 16)
                for b in range(1, 4):
                    sync.dma_start(out=yT_sb[:, b * kt:(b + 1) * kt, :],
                                   in_=yT_d[:, b * kt:(b + 1) * kt, :]
                                   ).then_inc(s_yb[b], 16)
                    yb_need[b] = 16
                sync.dma_start(out=cy_sb[:, 0:PRW],
                               in_=cy_d[:, 0:PRW]).then_inc(s_cy[0], 16)
                sync.dma_start(out=xT_sb[:, kt:4 * kt, :],
                               in_=xT_d[:, kt:4 * kt, :]).then_inc(s_xT[1], 16)
                sync.dma_start(out=xT_sb[:, 4 * kt:, :],
                               in_=xT_d[:, 4 * kt:, :]).then_inc(s_xT[2], 16)
                for q in range(1, nq):
                    sync.dma_start(out=cy_sb[:, q * PRW:(q + 1) * PRW],
                                   in_=cy_d[:, q * PRW:(q + 1) * PRW]
                                   ).then_inc(s_cy[q], 16)
                    for b in range(4 * q, 4 * q + 4):
                        sync.dma_start(out=yT_sb[:, b * kt:(b + 1) * kt, :],
                                       in_=yT_d[:, b * kt:(b + 1) * kt, :]
                                       ).then_inc(s_yb[b], 16)
                        yb_need[b] = 16
                # output drain; each chunk split across 2 queues
                for gq in range(GQ):
                    q, m = gq // mt, gq % mt
                    sl = gq % OS
                    msl = slice(m * P, (m + 1) * P)
                    osl_wait[gq] = osl_cnt[sl]
                    nsp = SPLIT if gq == GQ - 1 else 2
                    w = PRW // nsp
                    for c in range(nsp):
                        if gq == GQ - 1:
                            sync.wait_ge(s_dve, gq + c + 1)
                        elif c == 0:
                            sync.wait_ge(s_dve, gq + 1)
                        sync.dma_start(
                            out=out_d[msl, q * PRW + c * w:
                                      q * PRW + (c + 1) * w],
                            in_=o_sb[:, sl * PRW + c * w:
                                     sl * PRW + (c + 1) * w]
                        ).then_inc(s_osl[sl], 16)
                        osl_cnt[sl] += 16

            @block.tensor
            def _(tensor):
                tensor.wait_ge(s_scr, 1)
                for _ in range(NWARM):
                    tensor.matmul(ps[:, 0:P], lhsT=scr_sb[:, 0:2, :],
                                  rhs=scr_sb[:, 0:2, :], start=True,
                                  stop=True, perf_mode=DR,
                                  skip_group_check=True)
                tensor.wait_ge(s_xT[0], 16)
                for gq in range(GQ):
                    q, m = gq // mt, gq % mt
                    pr = gq % 2
                    if q == 0 and m == 1:
                        tensor.wait_ge(s_xT[1], 16)
                    if q == 0 and m == 4:
                        tensor.wait_ge(s_xT[2], 16)
                    # one psum-recycle wait covers both slots of the
                    # next two pairs (slot0 freed by ACT gq-2 <= gq-1,
                    # slot1 freed by ACT gq-1)
                    if gq >= 2 and gq % 2 == 0:
                        tensor.wait_ge(s_act, gq - 1)
                    for kp in range(2):
                        for j in range(2):
                            for nn in range(2):
                                b = 4 * q + 2 * j + nn
                                if m == 0 and kp == 0:
                                    tensor.wait_ge(s_yb[b], yb_need[b])
                                inst = tensor.matmul(
                                    ps[:, pr * PRW + j * 2 * NT + nn * NT:
                                       pr * PRW + j * 2 * NT + (nn + 1) * NT],
                                    lhsT=lhsT(m, kp),
                                    rhs=rhs(b, kp),
                                    start=(kp == 0),
                                    stop=(kp == 1),
                                    perf_mode=DR,
                                    skip_group_check=True,
                                )
                                if kp == 1 and gq == GQ - 1:
                                    inst.then_inc(s_mm, 1)
                    if gq < GQ - 1:
                        inst.then_inc(s_mm, 1)

            @block.scalar
            def _(scalar):
                # dummy exp on (garbage) SBUF to pull the ACT table load
                # into the DMA window; output lands in e_sb slot 0 which
                # the first real EXP overwrites
                scalar.activation(
                    e_sb[:, 0:2], e_sb[:, 2:4],
                    mybir.ActivationFunctionType.Exp,
                    bias=0.0, scale=0.0)
                scalar.wait_ge(s_x2, 16)
                for gq in range(GQ):
                    q, m = gq // mt, gq % mt
                    pr = gq % 2
                    se = gq % ES
                    if gq >= ES:
                        scalar.wait_ge(s_dve, gq - ES + 1)
                    if gq < GQ - 1:
                        scalar.wait_ge(s_mm, gq + 1)
                        scalar.activation(
                            e_sb[:, se * PRW:(se + 1) * PRW],
                            ps[:, pr * PRW:(pr + 1) * PRW],
                            mybir.ActivationFunctionType.Exp,
                            bias=x2_sb[:, m:m + 1],
                            scale=float(scale2g)).then_inc(s_act, 1)
                    else:
                        for c in range(SPLIT):
                            w = PRW // SPLIT
                            scalar.wait_ge(s_mm, gq + c + 1)
                            scalar.activation(
                                e_sb[:, se * PRW + c * w:se * PRW + (c + 1) * w],
                                ps[:, pr * PRW + c * w:pr * PRW + (c + 1) * w],
                                mybir.ActivationFunctionType.Exp,
                                bias=x2_sb[:, m:m + 1],
                                scale=float(scale2g)).then_inc(s_act, 1)

            @block.vector
            def _(vector):
                vector.memset(scr_sb[:], 0.0).then_inc(s_scr, 1)
                for gq in range(GQ):
                    q, m = gq // mt, gq % mt
                    se = gq % ES
                    sl = gq % OS
                    if m == 0:
                        vector.wait_ge(s_cy[q], 16)
                    if osl_wait[gq] > 0:
                        vector.wait_ge(s_osl[sl], osl_wait[gq])
                    if gq < GQ - 1:
                        vector.wait_ge(s_act, gq + 1)
                        vector.tensor_mul(
                            o_sb[:, sl * PRW:(sl + 1) * PRW],
                            e_sb[:, se * PRW:(se + 1) * PRW],
                            cy_sb[:, q * PRW:(q + 1) * PRW]).then_inc(s_dve, 1)
                    else:
                        for c in range(SPLIT):
                            w = PRW // SPLIT
                            vector.wait_ge(s_act, gq + c + 1)
                            vector.tensor_mul(
                                o_sb[:, sl * PRW + c * w:sl * PRW + (c + 1) * w],
                                e_sb[:, se * PRW + c * w:se * PRW + (c + 1) * w],
                                cy_sb[:, q * PRW + c * w:q * PRW + (c + 1) * w]
                            ).then_inc(s_dve, 1)

        nc.compile()
    return nc


def _pack_xT(x_b: np.ndarray) -> np.ndarray:
    """[MC, D] -> SBUF image [128, MT*KT, 128], block (m,k) at mid-index
    m*KT+k with element [p, ., c] = x[m*128 + c, k*128 + p]."""
    mcc, d = x_b.shape
    mt, kt = mcc // P, d // P
    a = x_b.reshape(mt, P, kt, P)          # [m, c, k, p]
    a = a.transpose(3, 0, 2, 1)            # [p, m, k, c]
    return np.ascontiguousarray(a.reshape(P, mt * kt, P))


def _pack_yT(y_b: np.ndarray, cw: int) -> np.ndarray:
    """[M, D] -> SBUF image [128, (M//cw)*KT, cw], block (b,k) at mid-index
    b*KT+k with element [p, ., c] = y[b*cw + c, k*128 + p]."""
    m, d = y_b.shape
    nbb, kt = m // cw, d // P
    a = y_b.reshape(nbb, cw, kt, P)        # [b, c, k, p]
    a = a.transpose(3, 0, 2, 1)            # [p, b, k, c]
    return np.ascontiguousarray(a.reshape(P, nbb * kt, cw))


def kernel(x: np.ndarray, y: np.ndarray, gamma: np.ndarray) -> np.ndarray:
    from concourse.bass_utils import run_bass_kernel_spmd

    x = np.asarray(x, dtype=np.float32)
    y = np.asarray(y, dtype=np.float32)
    g = float(np.asarray(gamma))

    n, d = x.shape
    m = y.shape[0]
    assert (n, d, m) == (N_FULL, D, M_FULL), (n, d, m)

    key = (g, n, d, m)
    if key not in _cache:
        _cache.clear()
        _cache[key] = _build_program_raw(2.0 * g, MC, M_FULL, D)
    nc = _cache[key]

    # host-side prep (O(N*D), ~0.01% of kernel FLOPs)
    f8 = ml_dtypes.float8_e4m3
    bf16 = ml_dtypes.bfloat16
    yTb = _pack_yT(y.astype(f8), NT)
    y2 = np.einsum("md,md->m", y, y, dtype=np.float64)
    cy_row = np.exp(-g * y2).astype(bf16)                       # [M]
    cyb = np.ascontiguousarray(np.broadcast_to(cy_row, (P, m)))
    x2 = np.einsum("nd,nd->n", x, x, dtype=np.float64)

    in_maps = []
    for c in range(N_CORES):
        sl = slice(c * MC, (c + 1) * MC)
        x2_c = np.ascontiguousarray(
            (-g * x2[sl]).astype(np.float32).reshape(MT, P).T)  # [128, MT]
        in_maps.append({"xTb": _pack_xT(x[sl].astype(f8)), "yTb": yTb,
                        "cyb": cyb, "x2b": x2_c})

    trace = bool(int(os.environ.get("RBF_TRACE", "0")))
    res = run_bass_kernel_spmd(nc, in_maps, core_ids=list(range(N_CORES)),
                               trace=trace)
    global LAST_RESULTS
    LAST_RESULTS = res
    return np.concatenate(
        [r["out"].astype(np.float32) for r in res.results], axis=0)


LAST_RESULTS = None
